# revision 29
# baseline (speedup 1.0000x reference)
"""StyleGAN2-style modulated conv (per-sample 3x3, 256->256 ch, 128x128) on 8 TRN2 cores.

Data-parallel over batch: core c computes sample c entirely on-chip.
Winograd F(2,3) along H cuts tensor-engine matmuls 1.5x vs direct
shift-and-matmul: per output-row pair, 4 transformed taps replace 6
direct taps.  The input transform (4 stride-2 row combinations) runs on
DVE in fp16 at 2x mode; the output transform (two adds per parity) also
on DVE; PSUM->SBUF m-tile eviction on ACT.  Demodulation is applied as a
per-output-channel scale on the assembled fp16 y tile (dn computed from
the modulated fp16 weights via a ones-column matmul, so style^2 is
already folded in).  Output is stored fp16 and widened on the host.
"""

import numpy as np
from contextlib import ExitStack

import concourse.bass as bass
import concourse.mybir as mybir
import concourse.tile as tile
from concourse import bacc
from concourse.masks import make_identity

FP32 = mybir.dt.float32
FP16 = mybir.dt.float16
AX = mybir.AxisListType
AF = mybir.ActivationFunctionType

B = 8
CI = 256
CO = 256
H = 128
W = 128
KS = 3
Z = 512
NKK = KS * KS          # 9 kernel taps
IT = CI // 128         # 2 input-channel tiles
OT = CO // 128         # 2 output-channel tiles
RG = 16                # output rows per group
G = H // RG            # 8 row groups
TY = RG // 2           # 8 Winograd row-pair tiles per group
WP = W + 4             # padded width: col0 = w=-1, 129 = w=128, 130-131 dead
                       # (132 keeps row stride 4B-aligned for DVE 2x mode)
EPS = 1e-8


def build_nc() -> bass.Bass:
    nc = bacc.Bacc("TRN2", target_bir_lowering=False, debug=False)
    x_d = nc.dram_tensor("x", [CI, H, W], FP32, kind="ExternalInput")
    w_d = nc.dram_tensor("w", [Z], FP32, kind="ExternalInput")
    wt_d = nc.dram_tensor("weight", [CO, CI, KS, KS], FP32, kind="ExternalInput")
    aw_d = nc.dram_tensor("affine_w", [CI, Z], FP32, kind="ExternalInput")
    ab_d = nc.dram_tensor("affine_b", [CI], FP32, kind="ExternalInput")
    y_d = nc.dram_tensor("y", [CO, H, W], FP16, kind="ExternalOutput")

    with tile.TileContext(nc) as tc, ExitStack() as ctx:
        singles = ctx.enter_context(tc.tile_pool(name="singles", bufs=1))
        work = ctx.enter_context(tc.tile_pool(name="work", bufs=2))
        cpool = ctx.enter_context(tc.tile_pool(name="cw", bufs=3))
        wopool = ctx.enter_context(tc.tile_pool(name="wo", bufs=2))
        xstage = ctx.enter_context(tc.tile_pool(name="xstage", bufs=4))
        xpool = ctx.enter_context(tc.tile_pool(name="xg", bufs=3))
        vpool = ctx.enter_context(tc.tile_pool(name="vg", bufs=4))
        epool = ctx.enter_context(tc.tile_pool(name="ev", bufs=8))
        ctpool = ctx.enter_context(tc.tile_pool(name="ct", bufs=4))
        ypool = ctx.enter_context(tc.tile_pool(name="yst", bufs=3))

        # ---- weight DMA first, split by i-tile half ([O, I*9] contiguous) ----
        wo = [
            wopool.tile([128, CI * NKK], FP32, name=f"wo{ot}", tag=f"wo{ot}")
            for ot in range(OT)
        ]
        HALF = (CI // IT) * NKK

        def load_wo_half(it):
            for ot in range(OT):
                nc.sync.dma_start(
                    out=wo[ot][:, it * HALF:(it + 1) * HALF],
                    in_=wt_d[
                        ot * 128:(ot + 1) * 128, it * 128:(it + 1) * 128
                    ].rearrange("o i kh kw -> o (i kh kw)"),
                )

        # ---- small input DMAs (style path) on the scalar queue so they land
        # ahead of the weight traffic on the sync queue ----
        wb = singles.tile([128, Z], FP32)
        w_ap = w_d[:]
        nc.scalar.dma_start(
            out=wb,
            in_=bass.AP(tensor=w_ap.tensor, offset=w_ap.offset, ap=[[0, 128], [1, Z]]),
        )
        af_b = singles.tile([128, IT, Z], FP32, tag="af")
        nc.scalar.dma_start(
            out=af_b, in_=aw_d.rearrange("(t p) z -> p t z", p=128)
        )
        ab_b = singles.tile([128, IT], FP32, tag="ab")
        nc.scalar.dma_start(
            out=ab_b, in_=ab_d.rearrange("(t p) -> p t", p=128)
        )
        af = [af_b[:, it, :] for it in range(IT)]
        ab1 = [ab_b[:, it:it + 1] for it in range(IT)]

        load_wo_half(0)
        load_wo_half(1)

        # ---- x row-group loads: DMA fp32 -> stage; ACT casts into zero-padded
        # fp16 xg; DVE zero-fills the pad columns/rows ----
        zrow = singles.tile([128, WP], FP16)
        nc.vector.memset(zrow, 0.0)

        xg_tiles: dict = {}

        def load_group(g: int):
            r0 = g * RG
            lo, hi = r0 - 1, r0 + RG + 1
            clo, chi = max(lo, 0), min(hi, H)
            nrows = chi - clo
            stgs = []
            for it in range(IT):
                stg = xstage.tile([128, RG + 2, W], FP32, name="stg", tag="stg")
                # spread x loads over hardware DMA queues; group 0 gets the
                # head slot of two otherwise-idle queues so casts start early
                if g == 0:
                    eng = nc.scalar if it == 0 else nc.gpsimd
                else:
                    eng = nc.sync if it == 0 else nc.gpsimd
                eng.dma_start(
                    out=stg[:, 0:nrows, :],
                    in_=x_d[it * 128:(it + 1) * 128, clo:chi, :],
                )
                stgs.append(stg)
            xg_tiles[g] = (stgs, clo, chi, lo, hi)

        xg_cast: dict = {}

        def cast_group(g: int, it: int):
            stgs, clo, chi, lo, hi = xg_tiles[g]
            nrows = chi - clo
            t = xpool.tile([128, RG + 2, WP], FP16, name="xg", tag="xg")
            # cast on GPSIMD: keeps ACT free for the PSUM eviction stream
            nc.gpsimd.tensor_copy(
                out=t[:, clo - lo: chi - lo, 1:W + 1], in_=stgs[it][:, 0:nrows, :]
            )
            nc.vector.tensor_copy(out=t[:, :, 0], in_=zrow[:, 0:RG + 2])
            nc.vector.tensor_copy(
                out=t[:, :, W + 1:WP],
                in_=zrow[:, 0:(RG + 2) * 3].rearrange("p (a b) -> p a b", b=3),
            )
            if lo < 0:
                nc.vector.tensor_copy(out=t[:, 0, :], in_=zrow)
            if hi > H:
                nc.vector.tensor_copy(out=t[:, RG + 1, :], in_=zrow)
            xg_cast.setdefault(g, {})[it] = t
            if len(xg_cast[g]) == IT:
                xg_tiles[g] = [xg_cast[g][0], xg_cast[g][1]]

        v_tiles: dict = {}

        def emit_v(g: int):
            # V_r row combinations (fp16, stride-2 row slices, DVE 2x mode)
            tiles = []
            for it in range(IT):
                xgt = xg_tiles[g][it]
                d0 = xgt[:, 0:2 * TY:2, :]
                d1 = xgt[:, 1:2 * TY + 1:2, :]
                d2 = xgt[:, 2:2 * TY + 2:2, :]
                d3 = xgt[:, 3:2 * TY + 2:2, :]
                v = vpool.tile([128, 4, TY, WP], FP16, name="vg", tag="vg")
                nc.vector.tensor_sub(v[:, 0], d0, d2)
                nc.vector.tensor_add(v[:, 1], d1, d2)
                nc.vector.tensor_sub(v[:, 2], d2, d1)
                nc.vector.tensor_sub(v[:, 3], d1, d3)
                tiles.append(v)
            v_tiles[g] = tiles

        load_group(0)
        load_group(1)

        # ---- ACT table pre-warm: force the activation-table load to happen
        # during the framework preamble, not in front of the first cast ----
        warm0 = singles.tile([128, 1], FP32, tag="warm0")
        nc.vector.memset(warm0, 0.0)
        warm1 = singles.tile([128, 1], FP32, tag="warm1")
        nc.scalar.mul(out=warm1, in_=warm0, mul=1.0)

        ident = singles.tile([128, 128], FP32)
        make_identity(nc, ident)
        eps_t = singles.tile([128, 1], FP32)
        nc.vector.memset(eps_t, EPS)
        ones_t = singles.tile([128, 1], FP32)
        nc.vector.memset(ones_t, 1.0)

        # ---- style columns: st[it] = w @ affine_w.T + affine_b + 1 ----
        st = []
        for it in range(IT):
            tmp = work.tile([128, Z], FP32, name="tmp", tag="styletmp")
            nc.vector.tensor_mul(tmp, af[it], wb)
            s = singles.tile([128, 1], FP32, name="s", tag=f"st{it}")
            nc.vector.reduce_sum(s, tmp, axis=AX.X)
            nc.vector.tensor_add(s, s, ab1[it])
            nc.vector.tensor_scalar_add(s, s, 1.0)
            st.append(s)

        # group-0 casts go FIRST in the ACT queue (ahead of the 36 weight
        # evictions) so V(g0) is ready by the time the weights are
        cast_group(0, 0)
        cast_group(0, 1)
        emit_v(0)

        # ---- PE transpose of weights; ACT evicts with style modulation to
        # fp16: wTm[it][i, kk, o] = weight[o, i, kk] * st[i].  DVE then builds
        # the Winograd-in-H combos r1/r2 = (W0 +/- W1 + W2)/2 and the
        # sum-of-squares path for demodulation. ----
        wTm = [
            singles.tile([128, NKK, CO], FP16, name=f"wTm{it}", tag=f"wTm{it}")
            for it in range(IT)
        ]
        wm12 = [
            singles.tile([128, KS, 2, CO], FP16, name=f"wm12{it}", tag=f"wm12{it}")
            for it in range(IT)
        ]
        q = [
            singles.tile([128, CO], FP32, name=f"q{it}", tag=f"q{it}")
            for it in range(IT)
        ]
        with tc.tile_pool(name="tpsum", bufs=4, space="PSUM") as tps:
            for it in range(IT):
                for ot in range(OT):
                    for kh in range(KS):
                        # 3 transposes (one kh row of taps) share one PSUM
                        # tile -> one modulated eviction (amortizes the ~200ns
                        # per-op ACT overhead)
                        pt = tps.tile([128, KS, 128], FP32, name="pt", tag="pt")
                        for kw in range(KS):
                            kk = kh * KS + kw
                            src = wo[ot].rearrange("o (i k) -> o i k", k=NKK)[
                                :, it * 128:(it + 1) * 128, kk
                            ]
                            nc.tensor.transpose(
                                out=pt[:, kw], in_=src, identity=ident
                            )
                        nc.scalar.mul(
                            out=wTm[it][:, kh * KS:(kh + 1) * KS,
                                        ot * 128:(ot + 1) * 128],
                            in_=pt,
                            mul=st[it],
                        )
                # Winograd weight combos, batched over all kw at once:
                # r1 = (W_kh0+W_kh1+W_kh2)/2, r2 = (W_kh0-W_kh1+W_kh2)/2
                s0 = wTm[it][:, 0 * KS:1 * KS, :]
                s1 = wTm[it][:, 1 * KS:2 * KS, :]
                s2 = wTm[it][:, 2 * KS:3 * KS, :]
                t02 = cpool.tile([128, KS, CO], FP16, name="t02", tag="t02")
                nc.vector.tensor_add(t02, s0, s2)
                u = cpool.tile([128, KS, CO], FP16, name="u", tag="u")
                nc.vector.tensor_add(u, t02, s1)
                v = cpool.tile([128, KS, CO], FP16, name="v", tag="v")
                nc.vector.tensor_sub(v, t02, s1)
                nc.vector.tensor_scalar_mul(wm12[it][:, :, 0, :], u, 0.5)
                nc.vector.tensor_scalar_mul(wm12[it][:, :, 1, :], v, 0.5)

        # demod sum-of-squares (after weight path on DVE)
        load_group(2)
        for it in range(IT):
            sqf = work.tile([128, NKK, CO], FP16, name="sqf", tag="sqf")
            nc.vector.tensor_mul(sqf, wTm[it], wTm[it])
            nc.vector.reduce_sum(
                q[it], sqf.rearrange("p k o -> p o k"), axis=AX.X
            )

        # ---- conv: Winograd-H, 8 groups x 2 ot x 2 halves; m_r tiles in
        # PSUM (1 bank each), r-major so evictions pipeline ----
        mpool = ctx.enter_context(tc.tile_pool(name="mp", bufs=7, space="PSUM"))
        pdpool = ctx.enter_context(tc.tile_pool(name="pdp", bufs=1, space="PSUM"))

        dn = []

        def emit_denom():
            # dn[ot] = 1/sqrt(sum_i q[i, o] + eps) as an O-column
            for ot in range(OT):
                pdt = pdpool.tile([128, 512], FP32, name="pd", tag="pd")
                pd = pdt[:, 0:1]
                for it in range(IT):
                    nc.tensor.matmul(
                        pd,
                        lhsT=q[it][:, ot * 128:(ot + 1) * 128],
                        rhs=ones_t,
                        start=(it == 0),
                        stop=(it == IT - 1),
                    )
                dcol = singles.tile([128, 1], FP32, name="dn", tag=f"dn{ot}")
                nc.scalar.activation(out=dcol, in_=pd, func=AF.Sqrt, bias=eps_t)
                nc.vector.reciprocal(dcol, dcol)
                dn.append(dcol)

        def half_unit(g: int, ot: int, half: int, ystage):
            hb = half * (RG // 2)  # first output row (of 16) in this half
            es = []
            for r in range(4):
                m = mpool.tile([128, 4, W], FP32, name="m", tag="m")
                mo = m.rearrange("p a w -> p (a w)")
                for it in range(IT):
                    for kw in range(KS):
                        if r == 0:
                            lhs = wTm[it][:, kw, ot * 128:(ot + 1) * 128]
                        elif r == 3:
                            lhs = wTm[it][:, 2 * KS + kw, ot * 128:(ot + 1) * 128]
                        else:
                            lhs = wm12[it][:, kw, r - 1, ot * 128:(ot + 1) * 128]
                        rhs = v_tiles[g][it][
                            :, r, half * 4: half * 4 + 4, kw:kw + W
                        ]
                        nc.tensor.matmul(
                            mo,
                            lhsT=lhs,
                            rhs=rhs,
                            start=(it == 0 and kw == 0),
                            stop=(it == IT - 1 and kw == KS - 1),
                        )
                e = epool.tile([128, 4, W], FP16, name="e", tag="e")
                nc.scalar.copy(out=e, in_=m)
                es.append(e)
            # output transform: y_even = (e0+e1)+e2, y_odd = (e1-e2)-e3
            te = ctpool.tile([128, 4, W], FP16, name="te", tag="te")
            nc.vector.tensor_add(te, es[0], es[1])
            nc.vector.tensor_add(ystage[:, hb + 0:hb + 8:2, :], te, es[2])
            to = ctpool.tile([128, 4, W], FP16, name="to", tag="to")
            nc.vector.tensor_sub(to, es[1], es[2])
            nc.vector.tensor_sub(ystage[:, hb + 1:hb + 8:2, :], to, es[3])

        def scale_and_store(g, ot, half, ystage):
            hb = half * (RG // 2)
            ys = ystage[:, hb:hb + 8, :]
            nc.vector.tensor_scalar_mul(ys, ys, dn[ot])
            nc.gpsimd.dma_start(
                out=y_d[ot * 128:(ot + 1) * 128,
                        g * RG + hb:g * RG + hb + 8, :],
                in_=ys,
            )

        for g in range(G):
            for ot in range(OT):
                ystage = ypool.tile([128, RG, W], FP16, name="yst", tag="yst")
                for half in range(2):
                    half_unit(g, ot, half, ystage)
                    # interleave prefetch / prep mid-group
                    if ot == 0 and g + 1 < G:
                        cast_group(g + 1, half)
                    if ot == 1 and half == 0:
                        if g + 1 < G:
                            emit_v(g + 1)
                        if g + 3 < G:
                            load_group(g + 3)
                    if g == 0 and ot == 0:
                        if half == 1:
                            emit_denom()
                            scale_and_store(g, ot, 0, ystage)
                            scale_and_store(g, ot, 1, ystage)
                    else:
                        scale_and_store(g, ot, half, ystage)
    nc.finalize()
    return nc


_CACHE: dict = {}


def _get_nc() -> bass.Bass:
    if "nc" not in _CACHE:
        _CACHE["nc"] = build_nc()
    return _CACHE["nc"]


def make_in_maps(x, w, weight, affine_w, affine_b):
    x = np.ascontiguousarray(x, dtype=np.float32)
    w = np.ascontiguousarray(w, dtype=np.float32)
    weight = np.ascontiguousarray(weight, dtype=np.float32)
    affine_w = np.ascontiguousarray(affine_w, dtype=np.float32)
    affine_b = np.ascontiguousarray(affine_b, dtype=np.float32)
    return [
        {
            "x": x[c],
            "w": w[c],
            "weight": weight,
            "affine_w": affine_w,
            "affine_b": affine_b,
        }
        for c in range(B)
    ]


def run_on_hw(inputs: dict, trace: bool = False, tmpdir: str | None = None):
    from concourse.bass_utils import run_bass_kernel_spmd

    nc = _get_nc()
    in_maps = make_in_maps(**inputs)
    res = run_bass_kernel_spmd(
        nc, in_maps, core_ids=list(range(B)), trace=trace, tmpdir=tmpdir
    )
    y = np.stack([r["y"] for r in res.results], axis=0).astype(np.float32)
    return y, res


def kernel(x, w, weight, affine_w, affine_b):
    y, _ = run_on_hw(
        dict(x=x, w=w, weight=weight, affine_w=affine_w, affine_b=affine_b)
    )
    return y


# revision 30
# speedup vs baseline: 1.5913x; 1.5913x over previous
"""StyleGAN2-style modulated conv (per-sample 3x3, 256->256 ch, 128x128) on 8 TRN2 cores.

Data-parallel over batch: core c computes sample c entirely on-chip.
Winograd F(2,3) along H cuts tensor-engine matmuls 1.5x vs direct
shift-and-matmul: per output-row pair, 4 transformed taps replace 6
direct taps.  The input transform (4 stride-2 row combinations) runs on
DVE in fp16 at 2x mode; the output transform (two adds per parity) also
on DVE; PSUM->SBUF m-tile eviction on ACT.  Demodulation is applied as a
per-output-channel scale on the assembled fp16 y tile (dn computed from
the modulated fp16 weights via a ones-column matmul, so style^2 is
already folded in).  Output is stored fp16 and widened on the host.
"""

import numpy as np
from contextlib import ExitStack

import concourse.bass as bass
import concourse.mybir as mybir
import concourse.tile as tile
from concourse import bacc
from concourse.masks import make_identity

FP32 = mybir.dt.float32
FP16 = mybir.dt.float16
AX = mybir.AxisListType
AF = mybir.ActivationFunctionType

B = 8
CI = 256
CO = 256
H = 128
W = 128
KS = 3
Z = 512
NKK = KS * KS          # 9 kernel taps
IT = CI // 128         # 2 input-channel tiles
OT = CO // 128         # 2 output-channel tiles
RG = 16                # output rows per group
G = H // RG            # 8 row groups
TY = RG // 2           # 8 Winograd row-pair tiles per group
WP = W + 4             # padded width: col0 = w=-1, 129 = w=128, 130-131 dead
                       # (132 keeps row stride 4B-aligned for DVE 2x mode)
EPS = 1e-8


def build_nc() -> bass.Bass:
    nc = bacc.Bacc("TRN2", target_bir_lowering=False, debug=False)
    x_d = nc.dram_tensor("x", [CI, H, W], FP32, kind="ExternalInput")
    w_d = nc.dram_tensor("w", [Z], FP32, kind="ExternalInput")
    wt_d = nc.dram_tensor("weight", [CO, CI, KS, KS], FP32, kind="ExternalInput")
    aw_d = nc.dram_tensor("affine_w", [CI, Z], FP32, kind="ExternalInput")
    ab_d = nc.dram_tensor("affine_b", [CI], FP32, kind="ExternalInput")
    y_d = nc.dram_tensor("y", [CO, H, W], FP16, kind="ExternalOutput")

    with tile.TileContext(nc) as tc, ExitStack() as ctx:
        singles = ctx.enter_context(tc.tile_pool(name="singles", bufs=1))
        work = ctx.enter_context(tc.tile_pool(name="work", bufs=2))
        cpool = ctx.enter_context(tc.tile_pool(name="cw", bufs=3))
        wopool = ctx.enter_context(tc.tile_pool(name="wo", bufs=2))
        xstage = ctx.enter_context(tc.tile_pool(name="xstage", bufs=4))
        xpool = ctx.enter_context(tc.tile_pool(name="xg", bufs=3))
        vpool = ctx.enter_context(tc.tile_pool(name="vg", bufs=4))
        epool = ctx.enter_context(tc.tile_pool(name="ev", bufs=8))
        ctpool = ctx.enter_context(tc.tile_pool(name="ct", bufs=4))
        ypool = ctx.enter_context(tc.tile_pool(name="yst", bufs=3))

        # ---- weight DMA first, split by i-tile half ([O, I*9] contiguous) ----
        wo = [
            wopool.tile([128, CI * NKK], FP32, name=f"wo{ot}", tag=f"wo{ot}")
            for ot in range(OT)
        ]
        HALF = (CI // IT) * NKK

        def load_wo_half(it):
            for ot in range(OT):
                nc.sync.dma_start(
                    out=wo[ot][:, it * HALF:(it + 1) * HALF],
                    in_=wt_d[
                        ot * 128:(ot + 1) * 128, it * 128:(it + 1) * 128
                    ].rearrange("o i kh kw -> o (i kh kw)"),
                )

        # ---- small input DMAs (style path) on the scalar queue so they land
        # ahead of the weight traffic on the sync queue ----
        wb = singles.tile([128, Z], FP32)
        w_ap = w_d[:]
        nc.scalar.dma_start(
            out=wb,
            in_=bass.AP(tensor=w_ap.tensor, offset=w_ap.offset, ap=[[0, 128], [1, Z]]),
        )
        af_b = singles.tile([128, IT, Z], FP32, tag="af")
        nc.scalar.dma_start(
            out=af_b, in_=aw_d.rearrange("(t p) z -> p t z", p=128)
        )
        ab_b = singles.tile([128, IT], FP32, tag="ab")
        nc.scalar.dma_start(
            out=ab_b, in_=ab_d.rearrange("(t p) -> p t", p=128)
        )
        af = [af_b[:, it, :] for it in range(IT)]
        ab1 = [ab_b[:, it:it + 1] for it in range(IT)]

        load_wo_half(0)
        load_wo_half(1)

        # ---- x row-group loads: DMA fp32 -> stage; ACT casts into zero-padded
        # fp16 xg; DVE zero-fills the pad columns/rows ----
        zrow = singles.tile([128, WP], FP16)
        nc.vector.memset(zrow, 0.0)

        xg_tiles: dict = {}

        def load_group(g: int):
            r0 = g * RG
            lo, hi = r0 - 1, r0 + RG + 1
            clo, chi = max(lo, 0), min(hi, H)
            nrows = chi - clo
            stgs = []
            for it in range(IT):
                stg = xstage.tile([128, RG + 2, W], FP32, name="stg", tag="stg")
                # spread x loads over hardware DMA queues; group 0 gets the
                # head slot of two otherwise-idle queues so casts start early
                if g == 0:
                    eng = nc.scalar if it == 0 else nc.gpsimd
                else:
                    eng = nc.sync if it == 0 else nc.gpsimd
                eng.dma_start(
                    out=stg[:, 0:nrows, :],
                    in_=x_d[it * 128:(it + 1) * 128, clo:chi, :],
                )
                stgs.append(stg)
            xg_tiles[g] = (stgs, clo, chi, lo, hi)

        xg_cast: dict = {}

        def cast_group(g: int, it: int):
            stgs, clo, chi, lo, hi = xg_tiles[g]
            nrows = chi - clo
            t = xpool.tile([128, RG + 2, WP], FP16, name="xg", tag="xg")
            nc.scalar.copy(
                out=t[:, clo - lo: chi - lo, 1:W + 1], in_=stgs[it][:, 0:nrows, :]
            )
            nc.vector.tensor_copy(out=t[:, :, 0], in_=zrow[:, 0:RG + 2])
            nc.vector.tensor_copy(
                out=t[:, :, W + 1:WP],
                in_=zrow[:, 0:(RG + 2) * 3].rearrange("p (a b) -> p a b", b=3),
            )
            if lo < 0:
                nc.vector.tensor_copy(out=t[:, 0, :], in_=zrow)
            if hi > H:
                nc.vector.tensor_copy(out=t[:, RG + 1, :], in_=zrow)
            xg_cast.setdefault(g, {})[it] = t
            if len(xg_cast[g]) == IT:
                xg_tiles[g] = [xg_cast[g][0], xg_cast[g][1]]

        v_tiles: dict = {}

        def emit_v(g: int):
            # V_r row combinations (fp16, stride-2 row slices, DVE 2x mode)
            tiles = []
            for it in range(IT):
                xgt = xg_tiles[g][it]
                d0 = xgt[:, 0:2 * TY:2, :]
                d1 = xgt[:, 1:2 * TY + 1:2, :]
                d2 = xgt[:, 2:2 * TY + 2:2, :]
                d3 = xgt[:, 3:2 * TY + 2:2, :]
                v = vpool.tile([128, 4, TY, WP], FP16, name="vg", tag="vg")
                nc.vector.tensor_sub(v[:, 0], d0, d2)
                nc.vector.tensor_add(v[:, 1], d1, d2)
                nc.vector.tensor_sub(v[:, 2], d2, d1)
                nc.vector.tensor_sub(v[:, 3], d1, d3)
                tiles.append(v)
            v_tiles[g] = tiles

        load_group(0)
        load_group(1)

        # ---- ACT table pre-warm: force the activation-table load to happen
        # during the framework preamble, not in front of the first cast ----
        warm0 = singles.tile([128, 1], FP32, tag="warm0")
        nc.vector.memset(warm0, 0.0)
        warm1 = singles.tile([128, 1], FP32, tag="warm1")
        nc.scalar.mul(out=warm1, in_=warm0, mul=1.0)

        ident = singles.tile([128, 128], FP32)
        make_identity(nc, ident)
        eps_t = singles.tile([128, 1], FP32)
        nc.vector.memset(eps_t, EPS)
        ones_t = singles.tile([128, 1], FP32)
        nc.vector.memset(ones_t, 1.0)

        # ---- style columns: st[it] = w @ affine_w.T + affine_b + 1 ----
        st = []
        for it in range(IT):
            tmp = work.tile([128, Z], FP32, name="tmp", tag="styletmp")
            nc.vector.tensor_mul(tmp, af[it], wb)
            s = singles.tile([128, 1], FP32, name="s", tag=f"st{it}")
            nc.vector.reduce_sum(s, tmp, axis=AX.X)
            nc.vector.tensor_add(s, s, ab1[it])
            nc.vector.tensor_scalar_add(s, s, 1.0)
            st.append(s)

        # group-0 casts go FIRST in the ACT queue (ahead of the 36 weight
        # evictions) so V(g0) is ready by the time the weights are
        cast_group(0, 0)
        cast_group(0, 1)
        emit_v(0)

        # ---- PE transpose of weights; ACT evicts with style modulation to
        # fp16: wTm[it][i, kk, o] = weight[o, i, kk] * st[i].  DVE then builds
        # the Winograd-in-H combos r1/r2 = (W0 +/- W1 + W2)/2 and the
        # sum-of-squares path for demodulation. ----
        wTm = [
            singles.tile([128, NKK, CO], FP16, name=f"wTm{it}", tag=f"wTm{it}")
            for it in range(IT)
        ]
        wm12 = [
            singles.tile([128, KS, 2, CO], FP16, name=f"wm12{it}", tag=f"wm12{it}")
            for it in range(IT)
        ]
        q = [
            singles.tile([128, CO], FP32, name=f"q{it}", tag=f"q{it}")
            for it in range(IT)
        ]
        with tc.tile_pool(name="tpsum", bufs=4, space="PSUM") as tps:
            for it in range(IT):
                for ot in range(OT):
                    for kh in range(KS):
                        # 3 transposes (one kh row of taps) share one PSUM
                        # tile -> one modulated eviction (amortizes the ~200ns
                        # per-op ACT overhead)
                        pt = tps.tile([128, KS, 128], FP32, name="pt", tag="pt")
                        for kw in range(KS):
                            kk = kh * KS + kw
                            src = wo[ot].rearrange("o (i k) -> o i k", k=NKK)[
                                :, it * 128:(it + 1) * 128, kk
                            ]
                            nc.tensor.transpose(
                                out=pt[:, kw], in_=src, identity=ident
                            )
                        nc.scalar.mul(
                            out=wTm[it][:, kh * KS:(kh + 1) * KS,
                                        ot * 128:(ot + 1) * 128],
                            in_=pt,
                            mul=st[it],
                        )
                # Winograd weight combos, batched over all kw at once:
                # r1 = (W_kh0+W_kh1+W_kh2)/2, r2 = (W_kh0-W_kh1+W_kh2)/2
                s0 = wTm[it][:, 0 * KS:1 * KS, :]
                s1 = wTm[it][:, 1 * KS:2 * KS, :]
                s2 = wTm[it][:, 2 * KS:3 * KS, :]
                t02 = cpool.tile([128, KS, CO], FP16, name="t02", tag="t02")
                nc.vector.tensor_add(t02, s0, s2)
                u = cpool.tile([128, KS, CO], FP16, name="u", tag="u")
                nc.vector.tensor_add(u, t02, s1)
                v = cpool.tile([128, KS, CO], FP16, name="v", tag="v")
                nc.vector.tensor_sub(v, t02, s1)
                nc.vector.tensor_scalar_mul(wm12[it][:, :, 0, :], u, 0.5)
                nc.vector.tensor_scalar_mul(wm12[it][:, :, 1, :], v, 0.5)

        # demod sum-of-squares (after weight path on DVE)
        load_group(2)
        for it in range(IT):
            sqf = work.tile([128, NKK, CO], FP16, name="sqf", tag="sqf")
            nc.vector.tensor_mul(sqf, wTm[it], wTm[it])
            nc.vector.reduce_sum(
                q[it], sqf.rearrange("p k o -> p o k"), axis=AX.X
            )

        # ---- conv: Winograd-H, 8 groups x 2 ot x 2 halves; m_r tiles in
        # PSUM (1 bank each), r-major so evictions pipeline ----
        mpool = ctx.enter_context(tc.tile_pool(name="mp", bufs=7, space="PSUM"))
        pdpool = ctx.enter_context(tc.tile_pool(name="pdp", bufs=1, space="PSUM"))

        dn = []

        def emit_denom():
            # dn[ot] = 1/sqrt(sum_i q[i, o] + eps) as an O-column
            for ot in range(OT):
                pdt = pdpool.tile([128, 512], FP32, name="pd", tag="pd")
                pd = pdt[:, 0:1]
                for it in range(IT):
                    nc.tensor.matmul(
                        pd,
                        lhsT=q[it][:, ot * 128:(ot + 1) * 128],
                        rhs=ones_t,
                        start=(it == 0),
                        stop=(it == IT - 1),
                    )
                dcol = singles.tile([128, 1], FP32, name="dn", tag=f"dn{ot}")
                nc.scalar.activation(out=dcol, in_=pd, func=AF.Sqrt, bias=eps_t)
                nc.vector.reciprocal(dcol, dcol)
                dn.append(dcol)

        def half_unit(g: int, ot: int, half: int, ystage):
            hb = half * (RG // 2)  # first output row (of 16) in this half
            es = []
            for r in range(4):
                m = mpool.tile([128, 4, W], FP32, name="m", tag="m")
                mo = m.rearrange("p a w -> p (a w)")
                for it in range(IT):
                    for kw in range(KS):
                        if r == 0:
                            lhs = wTm[it][:, kw, ot * 128:(ot + 1) * 128]
                        elif r == 3:
                            lhs = wTm[it][:, 2 * KS + kw, ot * 128:(ot + 1) * 128]
                        else:
                            lhs = wm12[it][:, kw, r - 1, ot * 128:(ot + 1) * 128]
                        rhs = v_tiles[g][it][
                            :, r, half * 4: half * 4 + 4, kw:kw + W
                        ]
                        nc.tensor.matmul(
                            mo,
                            lhsT=lhs,
                            rhs=rhs,
                            start=(it == 0 and kw == 0),
                            stop=(it == IT - 1 and kw == KS - 1),
                        )
                e = epool.tile([128, 4, W], FP16, name="e", tag="e")
                nc.scalar.copy(out=e, in_=m)
                es.append(e)
            # output transform: y_even = (e0+e1)+e2, y_odd = (e1-e2)-e3
            te = ctpool.tile([128, 4, W], FP16, name="te", tag="te")
            nc.vector.tensor_add(te, es[0], es[1])
            nc.vector.tensor_add(ystage[:, hb + 0:hb + 8:2, :], te, es[2])
            to = ctpool.tile([128, 4, W], FP16, name="to", tag="to")
            nc.vector.tensor_sub(to, es[1], es[2])
            nc.vector.tensor_sub(ystage[:, hb + 1:hb + 8:2, :], to, es[3])

        def scale_and_store(g, ot, half, ystage):
            hb = half * (RG // 2)
            ys = ystage[:, hb:hb + 8, :]
            nc.vector.tensor_scalar_mul(ys, ys, dn[ot])
            nc.gpsimd.dma_start(
                out=y_d[ot * 128:(ot + 1) * 128,
                        g * RG + hb:g * RG + hb + 8, :],
                in_=ys,
            )

        for g in range(G):
            for ot in range(OT):
                ystage = ypool.tile([128, RG, W], FP16, name="yst", tag="yst")
                for half in range(2):
                    half_unit(g, ot, half, ystage)
                    # interleave prefetch / prep mid-group
                    if ot == 0 and g + 1 < G:
                        cast_group(g + 1, half)
                    if ot == 1 and half == 0:
                        if g + 1 < G:
                            emit_v(g + 1)
                        if g + 3 < G:
                            load_group(g + 3)
                    if g == 0 and ot == 0:
                        if half == 1:
                            emit_denom()
                            scale_and_store(g, ot, 0, ystage)
                            scale_and_store(g, ot, 1, ystage)
                    else:
                        scale_and_store(g, ot, half, ystage)
    nc.finalize()
    return nc


_CACHE: dict = {}


def _get_nc() -> bass.Bass:
    if "nc" not in _CACHE:
        _CACHE["nc"] = build_nc()
    return _CACHE["nc"]


def make_in_maps(x, w, weight, affine_w, affine_b):
    x = np.ascontiguousarray(x, dtype=np.float32)
    w = np.ascontiguousarray(w, dtype=np.float32)
    weight = np.ascontiguousarray(weight, dtype=np.float32)
    affine_w = np.ascontiguousarray(affine_w, dtype=np.float32)
    affine_b = np.ascontiguousarray(affine_b, dtype=np.float32)
    return [
        {
            "x": x[c],
            "w": w[c],
            "weight": weight,
            "affine_w": affine_w,
            "affine_b": affine_b,
        }
        for c in range(B)
    ]


def run_on_hw(inputs: dict, trace: bool = False, tmpdir: str | None = None):
    from concourse.bass_utils import run_bass_kernel_spmd

    nc = _get_nc()
    in_maps = make_in_maps(**inputs)
    res = run_bass_kernel_spmd(
        nc, in_maps, core_ids=list(range(B)), trace=trace, tmpdir=tmpdir
    )
    y = np.stack([r["y"] for r in res.results], axis=0).astype(np.float32)
    return y, res


def kernel(x, w, weight, affine_w, affine_b):
    y, _ = run_on_hw(
        dict(x=x, w=w, weight=weight, affine_w=affine_w, affine_b=affine_b)
    )
    return y
